# revision 30
# baseline (speedup 1.0000x reference)
"""Trainium2 Bass kernel for nn_AttentionLayer (sparse_attention).

Computation (per reference):
    xf = x.reshape(B, C, S);  S = W*H = 4096
    q = xf @ Wq.T + bq            [B, C, 16]
    k = xf @ Wk.T + bk            [B, C, 16]
    kq[b] = q[b] @ k[b].T         [B, C, C]
    A = softmax(kq, axis=0)       (over the batch axis -- Softmax2d)
    out[b] = A[b].T @ xf[b]       [B, C, S]

Sharding: data-parallel over batch, 2 batches per core (8 cores).  The
axis-0 softmax couples cores only through the denominator sum_b exp(kq),
exchanged via a single bf16 AllReduce.

v3 design notes (on top of v2):
  * q and k share ONE stationary operand: W packed [wq | pad16 | wk] as
    [128, 48] per s-chunk -> one matmul per (sc, batch).  The q/k phase
    is DMA-bound (xT 8 MB at the ~180 GB/s contended per-core HBM rate).
  * E is bf16 end-to-end: exp writes bf16, the pair-sum is a pure-bf16
    DVE add (2-byte fast path), the AllReduce stays bf16.
  * Normalize is sliver-granular (oc, cc): converting 32 KB readbacks
    alternate between the sync and scalar DMA queues, reciprocal and the
    b0 multiply on DVE, b1 multiply on gpsimd.  The first out-matmul is
    gated only by the (oc0, cc0) sliver chain (~1.5us after AllReduce),
    not a full-width normalize.
  * Out-phase matmuls use the 16-bit 1024-wide moving operand (psum
    tiles span 2 banks), halving instruction count: 128 MMs x ~480ns
    instead of 256 x ~265ns.
  * v1/v2 discipline retained: fp16 GEMMs with fp32 PSUM accumulate,
    fp16 output upcast on host, exp-sum bounce DMAs issued on the sync
    queue ahead of the bulk xn DMAs, AllReduce output in Shared space.
"""

import os
import numpy as np

import concourse.mybir as mybir
import concourse.tile as tile
from concourse import bacc
from concourse.bass_utils import run_bass_kernel_spmd

B, C, S, D = 16, 512, 4096, 16
N_CORES = 8
B_LOC = B // N_CORES          # 2 batches per core
CC = C // 128                 # 4 i-chunks
OC = C // 128                 # 4 o-blocks
SC = S // 128                 # 32 s-chunks
WP = 48                       # packed weight cols: wq(16) | pad(16) | wk(16)
F32 = mybir.dt.float32
F16 = mybir.dt.float16
BF16 = mybir.dt.bfloat16

_CACHE = {}


def _build():
    nc = bacc.Bacc("TRN2", target_bir_lowering=False, debug=False,
                   num_devices=N_CORES)
    # xT grouped 4 s-chunks per DMA so each dma_start moves 1 MiB
    # (>=1 MiB per transfer reaches ~78% of HBM peak vs ~50% at 256 KB)
    xT_d = nc.dram_tensor("xT", [SC // 4, 128, 4 * B_LOC * C], F16,
                          kind="ExternalInput")
    xn_d = nc.dram_tensor("xn", [B_LOC, C, S], F16, kind="ExternalInput")
    w_d = nc.dram_tensor("wr", [128, SC * WP], F16, kind="ExternalInput")
    b_d = nc.dram_tensor("bqk", [D, 2], F32, kind="ExternalInput")
    out_d = nc.dram_tensor("out", [B_LOC, C, S], F16, kind="ExternalOutput")
    rg = [list(range(N_CORES))]

    cc_in = nc.dram_tensor("cc_in", [128, OC * CC * 128], BF16, kind="Internal")
    cc_out = nc.dram_tensor("cc_out", [128, OC * CC * 128], BF16,
                            kind="Internal", addr_space="Shared")
    warm_in = nc.dram_tensor("warm_in", [128, 16], BF16, kind="Internal")
    warm_out = nc.dram_tensor("warm_out", [128, 16], BF16,
                              kind="Internal", addr_space="Shared")

    with tile.TileContext(nc) as tc:
        with (
            tc.tile_pool(name="persist", bufs=1) as persist,
            tc.tile_pool(name="outsb", bufs=4) as outp,
        ):
            # ---- warm-up AllReduce: fires immediately (no data deps) and
            # completes during the xT load, absorbing the collective
            # first-call ncfw latency and syncing rank start skew so the
            # real AllReduce's mesh phase starts promptly ----
            nc.gpsimd.collective_compute(
                "AllReduce", mybir.AluOpType.add, replica_groups=rg,
                ins=[warm_in.ap()], outs=[warm_out.ap()])

            # ---- constants ----
            wqk = persist.tile([128, SC, WP], F16, tag="wqk", name="wqk")
            nc.sync.dma_start(
                out=wqk, in_=w_d.ap().rearrange("p (n d) -> p n d", n=SC))
            bqk = persist.tile([D, 2], F32, tag="bqk", name="bqk")
            nc.sync.dma_start(out=bqk, in_=b_d.ap())

            # ---- x DMAs: xT first (gates q/k -> exp -> AllReduce) ----
            # flat 2D tiles: per-partition 8 KB contiguous on both sides so
            # the DMA emits 8 KB descriptors, not 4x2KB
            xT_sb = [persist.tile([128, 4 * B_LOC * C], F16, tag=f"xT{g}",
                                  name=f"xT{g}") for g in range(SC // 4)]
            # alternate the two HWDGE rings (qSPDynamicHW / qActDynamicHW)
            # so consecutive 1 MiB transfers overlap instead of serializing
            for g in range(SC // 4):
                rq = nc.sync if g % 2 == 0 else nc.scalar
                rq.dma_start(out=xT_sb[g], in_=xT_d.ap()[g])
            xn_sb = [[persist.tile([128, S], F16, tag=f"xn{b}_{cc}",
                                   name=f"xn{b}_{cc}") for cc in range(CC)]
                     for b in range(B_LOC)]

            q_sb = [persist.tile([D, C], F16, tag=f"q{b}", name=f"q{b}")
                    for b in range(B_LOC)]
            k_sb = [persist.tile([D, C], F16, tag=f"k{b}", name=f"k{b}")
                    for b in range(B_LOC)]
            # E is cc-major so exp writes contiguous [128, 512]; everything
            # downstream reads (oc, cc) slivers either way
            E_sb = [persist.tile([128, CC, OC, 128], BF16, tag=f"E{b}",
                                 name=f"E{b}") for b in range(B_LOC)]
            A_sb = [persist.tile([128, OC, CC, 128], F16, tag=f"A{b}",
                                 name=f"A{b}") for b in range(B_LOC)]
            Sl_sb = persist.tile([128, OC, CC, 128], BF16, tag="Sl", name="Sl")
            Sb_sb = persist.tile([128, OC, CC, 128], BF16, tag="Sb", name="Sb")
            Sf_sb = persist.tile([128, OC, CC, 128], F32, tag="Sf", name="Sf")
            R_sb = persist.tile([128, OC, CC, 128], F32, tag="R", name="R")

            # ---- q/k: one packed matmul per (sc, b) ----
            with (
                tc.tile_pool(name="ps_qk", bufs=2, space="PSUM") as ps_qk,
                tc.tile_pool(name="ps_kq", bufs=4, space="PSUM") as ps_kq,
            ):
                qk_ps = [ps_qk.tile([WP, C], F32, tag="qkps", name=f"qkps{i}")
                         for i in range(B_LOC)]
                for sc in range(SC):
                    for b in range(B_LOC):
                        nc.tensor.matmul(
                            qk_ps[b],
                            lhsT=wqk[:, sc, :],
                            rhs=xT_sb[sc // 4][:,
                                               (sc % 4) * B_LOC * C + b * C:
                                               (sc % 4) * B_LOC * C
                                               + (b + 1) * C],
                            start=(sc == 0), stop=(sc == SC - 1))
                # bias-evacs on DVE, batch 0 first so its kq can start early
                for b in range(B_LOC):
                    nc.vector.tensor_scalar_add(q_sb[b], qk_ps[b][0:D],
                                                bqk[:, 0:1])
                    nc.vector.tensor_scalar_add(k_sb[b], qk_ps[b][32:32 + D],
                                                bqk[:, 1:2])

                # ---- kq -> exp (contiguous cc-major bf16 writes) ----
                for b in range(B_LOC):
                    for cc in range(CC):
                        kq_ps = ps_kq.tile([128, C], F32)
                        nc.tensor.matmul(
                            kq_ps,
                            lhsT=q_sb[b][:, cc * 128:(cc + 1) * 128],
                            rhs=k_sb[b], start=True, stop=True)
                        nc.scalar.activation(
                            out=E_sb[b][:, cc], in_=kq_ps,
                            func=mybir.ActivationFunctionType.Exp)
                # pure-bf16 pair-sums on DVE; bounce each o-block as it lands
                for oc in range(OC):
                    nc.vector.tensor_add(Sl_sb[:, oc],
                                         E_sb[0][:, :, oc, :],
                                         E_sb[1][:, :, oc, :])
                    rq = nc.sync if oc % 2 == 0 else nc.scalar
                    rq.dma_start(
                        out=cc_in.ap()[:, oc * CC * 128:(oc + 1) * CC * 128],
                        in_=Sl_sb[:, oc])
                for bb in range(B_LOC):
                    for cc2 in range(CC):
                        rq = nc.sync if cc2 % 2 == 0 else nc.scalar
                        rq.dma_start(
                            out=xn_sb[bb][cc2],
                            in_=xn_d.ap()[bb, cc2 * 128:(cc2 + 1) * 128, :])

            # ---- single bf16 AllReduce of the local exp-sums ----
            nc.gpsimd.collective_compute(
                "AllReduce", mybir.AluOpType.add, replica_groups=rg,
                ins=[cc_in.ap()], outs=[cc_out.ap()])

            # ---- sliver normalize: (oc, cc) granular so the first
            # out-matmul unblocks ~1.5us after the AllReduce ----
            for oc in range(OC):
                for cc in range(CC):
                    col = (oc * CC + cc) * 128
                    rq = nc.sync if (oc * CC + cc) % 2 == 0 else nc.scalar
                    rq.dma_start(out=Sb_sb[:, oc, cc],
                                 in_=cc_out.ap()[:, col:col + 128])
                    nc.scalar.copy(Sf_sb[:, oc, cc], Sb_sb[:, oc, cc])
                    nc.vector.reciprocal_approx_fast(R_sb[:, oc, cc],
                                                     Sf_sb[:, oc, cc])
                    nc.vector.tensor_mul(A_sb[0][:, oc, cc],
                                         E_sb[0][:, cc, oc],
                                         R_sb[:, oc, cc])
                    nc.gpsimd.tensor_mul(A_sb[1][:, oc, cc],
                                         E_sb[1][:, cc, oc],
                                         R_sb[:, oc, cc])

            # ---- out[b] = A[b].T @ x[b]; o-block outer so block 0 can
            # start as soon as its normalize lands ----
            with tc.tile_pool(name="ps_out", bufs=8, space="PSUM") as ps_out:
                for oc in range(OC):
                    for b in range(B_LOC):
                        for h in range(2):
                            outps = [ps_out.tile([128, 512], F32,
                                                 tag="outps",
                                                 name=f"outps{j}")
                                     for j in range(4)]
                            for ic in range(CC):
                                for j in range(4):
                                    nc.tensor.matmul(
                                        outps[j],
                                        lhsT=A_sb[b][:, oc, ic, :],
                                        rhs=xn_sb[b][ic][:,
                                                         (h * 4 + j) * 512:
                                                         (h * 4 + j + 1) * 512],
                                        start=(ic == 0), stop=(ic == CC - 1))
                            o_sb = outp.tile([128, 2048], F16)
                            for j in range(4):
                                osl = slice(j * 512, (j + 1) * 512)
                                if j % 2 == 0:
                                    nc.vector.tensor_copy(o_sb[:, osl],
                                                          outps[j])
                                else:
                                    nc.scalar.copy(o_sb[:, osl], outps[j])
                            oq = nc.sync if (oc * 4 + b * 2 + h) % 2 == 0 \
                                else nc.scalar
                            oq.dma_start(
                                out=out_d.ap()[b,
                                               oc * 128:(oc + 1) * 128,
                                               h * 2048:(h + 1) * 2048],
                                in_=o_sb)
    nc.compile()
    return nc


def kernel(x, Wq, bq, Wk, bk):
    b_, c_, w_, h_ = x.shape
    xf16 = np.ascontiguousarray(
        x.reshape(b_, c_, w_ * h_), dtype=np.float16)           # [B, C, S]
    xT16 = np.ascontiguousarray(xf16.transpose(0, 2, 1))        # [B, S, C]
    # packed weight: per s-row [wq(16) | zeros(16) | wk(16)]
    wqk = np.zeros((S, WP), dtype=np.float16)
    wqk[:, 0:D] = Wq.T.astype(np.float16)
    wqk[:, 32:32 + D] = Wk.T.astype(np.float16)
    # [S, WP] -> [128, SC*WP] so the weight DMA is contiguous per partition
    w_r = np.ascontiguousarray(
        wqk.reshape(SC, 128, WP).transpose(1, 0, 2).reshape(128, SC * WP))
    bqk = np.stack([bq, bk], axis=1).astype(np.float32)  # [D, 2]

    if "nc" not in _CACHE:
        _CACHE["nc"] = _build()
    nc = _CACHE["nc"]

    in_maps = []
    for j in range(N_CORES):
        # [SC, 128, B_LOC*C] chunk-major, then group 4 chunks per
        # partition line so each 1 MiB DMA reads contiguous DRAM
        xTc = xT16[B_LOC * j: B_LOC * (j + 1)].transpose(1, 0, 2).reshape(
            SC, 128, B_LOC * C)
        xTg = np.ascontiguousarray(
            xTc.reshape(SC // 4, 4, 128, B_LOC * C).transpose(0, 2, 1, 3)
            .reshape(SC // 4, 128, 4 * B_LOC * C))  # per-partition 8KB runs
        in_maps.append(
            {"xT": xTg,
             "xn": np.ascontiguousarray(xf16[B_LOC * j: B_LOC * (j + 1)]),
             "wr": w_r, "bqk": bqk})
    trace = bool(int(os.environ.get("BASSKERNEL_TRACE", "0")))
    res = run_bass_kernel_spmd(nc, in_maps, core_ids=list(range(N_CORES)),
                               trace=trace)
    _CACHE["last_result"] = res
    out = np.concatenate([r["out"] for r in res.results], axis=0)
    return out.astype(np.float32).reshape(b_, c_, w_, h_)


# revision 32
# speedup vs baseline: 1.1452x; 1.1452x over previous
"""Trainium2 Bass kernel for nn_AttentionLayer (sparse_attention).

Computation (per reference):
    xf = x.reshape(B, C, S);  S = W*H = 4096
    q = xf @ Wq.T + bq            [B, C, 16]
    k = xf @ Wk.T + bk            [B, C, 16]
    kq[b] = q[b] @ k[b].T         [B, C, C]
    A = softmax(kq, axis=0)       (over the batch axis -- Softmax2d)
    out[b] = A[b].T @ xf[b]       [B, C, S]

Sharding: data-parallel over batch, 2 batches per core (8 cores).  The
axis-0 softmax couples cores only through the denominator sum_b exp(kq),
exchanged via a single bf16 AllReduce.

v3 design notes (on top of v2):
  * q and k share ONE stationary operand: W packed [wq | pad16 | wk] as
    [128, 48] per s-chunk -> one matmul per (sc, batch).  The q/k phase
    is DMA-bound (xT 8 MB at the ~180 GB/s contended per-core HBM rate).
  * E is bf16 end-to-end: exp writes bf16, the pair-sum is a pure-bf16
    DVE add (2-byte fast path), the AllReduce stays bf16.
  * Normalize is sliver-granular (oc, cc): converting 32 KB readbacks
    alternate between the sync and scalar DMA queues, reciprocal and the
    b0 multiply on DVE, b1 multiply on gpsimd.  The first out-matmul is
    gated only by the (oc0, cc0) sliver chain (~1.5us after AllReduce),
    not a full-width normalize.
  * Out-phase matmuls use the 16-bit 1024-wide moving operand (psum
    tiles span 2 banks), halving instruction count: 128 MMs x ~480ns
    instead of 256 x ~265ns.
  * v1/v2 discipline retained: fp16 GEMMs with fp32 PSUM accumulate,
    fp16 output upcast on host, exp-sum bounce DMAs issued on the sync
    queue ahead of the bulk xn DMAs, AllReduce output in Shared space.
"""

import os
import numpy as np

import concourse.mybir as mybir
import concourse.tile as tile
from concourse import bacc
from concourse.bass_utils import run_bass_kernel_spmd

B, C, S, D = 16, 512, 4096, 16
N_CORES = 8
B_LOC = B // N_CORES          # 2 batches per core
CC = C // 128                 # 4 i-chunks
OC = C // 128                 # 4 o-blocks
SC = S // 128                 # 32 s-chunks
WP = 48                       # packed weight cols: wq(16) | pad(16) | wk(16)
F32 = mybir.dt.float32
F16 = mybir.dt.float16
BF16 = mybir.dt.bfloat16

_CACHE = {}


def _build():
    nc = bacc.Bacc("TRN2", target_bir_lowering=False, debug=False,
                   num_devices=N_CORES)
    # xT grouped 4 s-chunks per DMA so each dma_start moves 1 MiB
    # (>=1 MiB per transfer reaches ~78% of HBM peak vs ~50% at 256 KB)
    xT_d = nc.dram_tensor("xT", [SC // 4, 128, 4 * B_LOC * C], F16,
                          kind="ExternalInput")
    xn_d = nc.dram_tensor("xn", [B_LOC, C, S], F16, kind="ExternalInput")
    w_d = nc.dram_tensor("wr", [128, SC * WP], F16, kind="ExternalInput")
    b_d = nc.dram_tensor("bqk", [D, 2], F32, kind="ExternalInput")
    out_d = nc.dram_tensor("out", [B_LOC, C, S], F16, kind="ExternalOutput")
    rg = [list(range(N_CORES))]

    cc_in = nc.dram_tensor("cc_in", [128, OC * CC * 128], BF16, kind="Internal")
    cc_out = nc.dram_tensor("cc_out", [128, OC * CC * 128], BF16,
                            kind="Internal", addr_space="Shared")


    with tile.TileContext(nc) as tc:
        with (
            tc.tile_pool(name="persist", bufs=1) as persist,
            tc.tile_pool(name="outsb", bufs=4) as outp,
        ):
            # ---- constants ----
            wqk = persist.tile([128, SC, WP], F16, tag="wqk", name="wqk")
            nc.sync.dma_start(
                out=wqk, in_=w_d.ap().rearrange("p (n d) -> p n d", n=SC))
            bqk = persist.tile([D, 2], F32, tag="bqk", name="bqk")
            nc.sync.dma_start(out=bqk, in_=b_d.ap())

            # ---- x DMAs: xT first (gates q/k -> exp -> AllReduce) ----
            # flat 2D tiles: per-partition 8 KB contiguous on both sides so
            # the DMA emits 8 KB descriptors, not 4x2KB
            xT_sb = [persist.tile([128, 4 * B_LOC * C], F16, tag=f"xT{g}",
                                  name=f"xT{g}") for g in range(SC // 4)]
            # alternate the two HWDGE rings (qSPDynamicHW / qActDynamicHW)
            # so consecutive 1 MiB transfers overlap instead of serializing
            for g in range(SC // 4):
                rq = nc.sync if g % 2 == 0 else nc.scalar
                rq.dma_start(out=xT_sb[g], in_=xT_d.ap()[g])
            xn_sb = [[persist.tile([128, S], F16, tag=f"xn{b}_{cc}",
                                   name=f"xn{b}_{cc}") for cc in range(CC)]
                     for b in range(B_LOC)]

            q_sb = [persist.tile([D, C], F16, tag=f"q{b}", name=f"q{b}")
                    for b in range(B_LOC)]
            k_sb = [persist.tile([D, C], F16, tag=f"k{b}", name=f"k{b}")
                    for b in range(B_LOC)]
            # E is cc-major so exp writes contiguous [128, 512]; everything
            # downstream reads (oc, cc) slivers either way
            E_sb = [persist.tile([128, CC, OC, 128], BF16, tag=f"E{b}",
                                 name=f"E{b}") for b in range(B_LOC)]
            A_sb = [persist.tile([128, OC, CC, 128], F16, tag=f"A{b}",
                                 name=f"A{b}") for b in range(B_LOC)]
            Sl_sb = persist.tile([128, OC, CC, 128], BF16, tag="Sl", name="Sl")
            Sb_sb = persist.tile([128, OC, CC, 128], BF16, tag="Sb", name="Sb")
            Sf_sb = persist.tile([128, OC, CC, 128], F32, tag="Sf", name="Sf")
            R_sb = persist.tile([128, OC, CC, 128], F32, tag="R", name="R")

            # ---- q/k: one packed matmul per (sc, b) ----
            with (
                tc.tile_pool(name="ps_qk", bufs=2, space="PSUM") as ps_qk,
                tc.tile_pool(name="ps_kq", bufs=4, space="PSUM") as ps_kq,
            ):
                qk_ps = [ps_qk.tile([WP, C], F32, tag="qkps", name=f"qkps{i}")
                         for i in range(B_LOC)]
                for sc in range(SC):
                    for b in range(B_LOC):
                        nc.tensor.matmul(
                            qk_ps[b],
                            lhsT=wqk[:, sc, :],
                            rhs=xT_sb[sc // 4][:,
                                               (sc % 4) * B_LOC * C + b * C:
                                               (sc % 4) * B_LOC * C
                                               + (b + 1) * C],
                            start=(sc == 0), stop=(sc == SC - 1))
                # bias-evacs on DVE, batch 0 first so its kq can start early
                for b in range(B_LOC):
                    nc.vector.tensor_scalar_add(q_sb[b], qk_ps[b][0:D],
                                                bqk[:, 0:1])
                    nc.vector.tensor_scalar_add(k_sb[b], qk_ps[b][32:32 + D],
                                                bqk[:, 1:2])

                # ---- kq -> exp (contiguous cc-major bf16 writes) ----
                for b in range(B_LOC):
                    for cc in range(CC):
                        kq_ps = ps_kq.tile([128, C], F32)
                        nc.tensor.matmul(
                            kq_ps,
                            lhsT=q_sb[b][:, cc * 128:(cc + 1) * 128],
                            rhs=k_sb[b], start=True, stop=True)
                        nc.scalar.activation(
                            out=E_sb[b][:, cc], in_=kq_ps,
                            func=mybir.ActivationFunctionType.Exp)
                # pure-bf16 pair-sums on DVE; bounce each o-block as it lands
                for oc in range(OC):
                    nc.vector.tensor_add(Sl_sb[:, oc],
                                         E_sb[0][:, :, oc, :],
                                         E_sb[1][:, :, oc, :])
                    rq = nc.sync if oc % 2 == 0 else nc.scalar
                    rq.dma_start(
                        out=cc_in.ap()[:, oc * CC * 128:(oc + 1) * CC * 128],
                        in_=Sl_sb[:, oc])
                for bb in range(B_LOC):
                    for cc2 in range(CC):
                        rq = nc.sync if cc2 % 2 == 0 else nc.scalar
                        rq.dma_start(
                            out=xn_sb[bb][cc2],
                            in_=xn_d.ap()[bb, cc2 * 128:(cc2 + 1) * 128, :])

            # ---- single bf16 AllReduce of the local exp-sums ----
            nc.gpsimd.collective_compute(
                "AllReduce", mybir.AluOpType.add, replica_groups=rg,
                ins=[cc_in.ap()], outs=[cc_out.ap()])

            # ---- sliver normalize: (oc, cc) granular so the first
            # out-matmul unblocks ~1.5us after the AllReduce ----
            for oc in range(OC):
                for cc in range(CC):
                    col = (oc * CC + cc) * 128
                    rq = nc.sync if (oc * CC + cc) % 2 == 0 else nc.scalar
                    rq.dma_start(out=Sb_sb[:, oc, cc],
                                 in_=cc_out.ap()[:, col:col + 128])
                    nc.scalar.copy(Sf_sb[:, oc, cc], Sb_sb[:, oc, cc])
                    nc.vector.reciprocal_approx_fast(R_sb[:, oc, cc],
                                                     Sf_sb[:, oc, cc])
                    nc.vector.tensor_mul(A_sb[0][:, oc, cc],
                                         E_sb[0][:, cc, oc],
                                         R_sb[:, oc, cc])
                    nc.gpsimd.tensor_mul(A_sb[1][:, oc, cc],
                                         E_sb[1][:, cc, oc],
                                         R_sb[:, oc, cc])

            # ---- out[b] = A[b].T @ x[b]; o-block outer so block 0 can
            # start as soon as its normalize lands ----
            with tc.tile_pool(name="ps_out", bufs=8, space="PSUM") as ps_out:
                for oc in range(OC):
                    for b in range(B_LOC):
                        for h in range(2):
                            outps = [ps_out.tile([128, 512], F32,
                                                 tag="outps",
                                                 name=f"outps{j}")
                                     for j in range(4)]
                            for ic in range(CC):
                                for j in range(4):
                                    nc.tensor.matmul(
                                        outps[j],
                                        lhsT=A_sb[b][:, oc, ic, :],
                                        rhs=xn_sb[b][ic][:,
                                                         (h * 4 + j) * 512:
                                                         (h * 4 + j + 1) * 512],
                                        start=(ic == 0), stop=(ic == CC - 1))
                            o_sb = outp.tile([128, 2048], F16)
                            for j in range(4):
                                osl = slice(j * 512, (j + 1) * 512)
                                if j % 2 == 0:
                                    nc.vector.tensor_copy(o_sb[:, osl],
                                                          outps[j])
                                else:
                                    nc.scalar.copy(o_sb[:, osl], outps[j])
                            oq = nc.sync if (oc * 4 + b * 2 + h) % 2 == 0 \
                                else nc.scalar
                            oq.dma_start(
                                out=out_d.ap()[b,
                                               oc * 128:(oc + 1) * 128,
                                               h * 2048:(h + 1) * 2048],
                                in_=o_sb)
    nc.compile()
    return nc


def kernel(x, Wq, bq, Wk, bk):
    b_, c_, w_, h_ = x.shape
    xf16 = np.ascontiguousarray(
        x.reshape(b_, c_, w_ * h_), dtype=np.float16)           # [B, C, S]
    xT16 = np.ascontiguousarray(xf16.transpose(0, 2, 1))        # [B, S, C]
    # packed weight: per s-row [wq(16) | zeros(16) | wk(16)]
    wqk = np.zeros((S, WP), dtype=np.float16)
    wqk[:, 0:D] = Wq.T.astype(np.float16)
    wqk[:, 32:32 + D] = Wk.T.astype(np.float16)
    # [S, WP] -> [128, SC*WP] so the weight DMA is contiguous per partition
    w_r = np.ascontiguousarray(
        wqk.reshape(SC, 128, WP).transpose(1, 0, 2).reshape(128, SC * WP))
    bqk = np.stack([bq, bk], axis=1).astype(np.float32)  # [D, 2]

    if "nc" not in _CACHE:
        _CACHE["nc"] = _build()
    nc = _CACHE["nc"]

    in_maps = []
    for j in range(N_CORES):
        # [SC, 128, B_LOC*C] chunk-major, then group 4 chunks per
        # partition line so each 1 MiB DMA reads contiguous DRAM
        xTc = xT16[B_LOC * j: B_LOC * (j + 1)].transpose(1, 0, 2).reshape(
            SC, 128, B_LOC * C)
        xTg = np.ascontiguousarray(
            xTc.reshape(SC // 4, 4, 128, B_LOC * C).transpose(0, 2, 1, 3)
            .reshape(SC // 4, 128, 4 * B_LOC * C))  # per-partition 8KB runs
        in_maps.append(
            {"xT": xTg,
             "xn": np.ascontiguousarray(xf16[B_LOC * j: B_LOC * (j + 1)]),
             "wr": w_r, "bqk": bqk})
    trace = bool(int(os.environ.get("BASSKERNEL_TRACE", "0")))
    res = run_bass_kernel_spmd(nc, in_maps, core_ids=list(range(N_CORES)),
                               trace=trace)
    _CACHE["last_result"] = res
    out = np.concatenate([r["out"] for r in res.results], axis=0)
    return out.astype(np.float32).reshape(b_, c_, w_, h_)
